# revision 27
# baseline (speedup 1.0000x reference)
"""BNT Channel Attention kernel for 8x TRN2 NeuronCores.

Reference computation (per batch b of 8, one batch per core):
    qkv = x @ W_qkv + b_qkv            # [4096, 3072]
    q, k, v = split(qkv)               # each [4096, 1024], 16 heads x 64
    attn_h = softmax((q_h^T @ k_h) / 8, axis=-1)   # [64, 64] per head
    out_h  = v_h @ attn_h              # [4096, 64]
    out    = concat_h(out_h)           # [4096, 1024]

Strategy:
- Data parallel over batch: core c handles batch c (no collectives).
- All matmuls in float32r (fp32 storage, 11-bit-mantissa PE mode, 1 cyc/row
  vs 4 for fp32). End-to-end absmax-rel error vs the f32 reference ~1.2e-3.
- The projection contracts over D, so x must be D-on-partitions: x is
  transposed on the HOST (numpy) and fed as xt [1024, 4096]; on-chip it is
  streamed per 512-row chunk with a single cast-DMA (f32 -> f32r).
- Pass A per chunk: QK projection (N=512 matmuls, PSUM k-accumulation),
  per-head-pair attention partials (closed per row-tile, accumulated in
  SBUF by DVE - long-lived PSUM accumulation is unsafe under the Tile
  scheduler). Attention matmuls are N=256 wide (two pairs of k-cols,
  junk half dropped) to dodge the f32r <256-wide 4x weight-load penalty.
- Softmax: batched over all 8 head pairs, writes a block-diagonal
  [128, 8*128] f32r tile bd (pair p diag blocks at cols 128p..).
- V path by associativity: out = x @ (Wv @ attn) + (bv @ attn).
  Wv' = Wv @ attn is tiny (host-pretransposed WvT input); pass B is then
  a plain second projection out = x @ Wv' re-streaming xT (PE-dense, no
  vT spill), with the bias folded in as rank-1 ones x bv' matmuls.
"""

import numpy as np

import concourse.bacc as bacc
import concourse.bass as bass
import concourse.mybir as mybir
import concourse.tile as tile
from concourse import bass_utils

B = 8
NSEQ = 4096
D = 1024
ND3 = 3 * D
H = 16
DH = 64
NPAIR = 8          # head pairs (2 heads = 128 channels per pair)
P = 128            # partitions / tile edge
KT = D // P        # 8 k-tiles over the contraction dim
MT = NSEQ // P     # 32 row-tiles
CHUNK = 512        # rows per chunk
NCHUNK = NSEQ // CHUNK
MPC = CHUNK // P   # 4 row-tiles per chunk

F32 = mybir.dt.float32
F32R = mybir.dt.float32r

_CACHE = {}
_LAST_RESULTS = None


def _build(xt_bufs=2):
    nc = bacc.Bacc(
        "TRN2", target_bir_lowering=False, debug=False, num_devices=B
    )
    xt_d = nc.dram_tensor("xt", [D, NSEQ], F32, kind="ExternalInput").ap()
    w_d = nc.dram_tensor("w", [D, ND3], F32, kind="ExternalInput").ap()
    bqk_d = nc.dram_tensor("bqk", [P, 2 * D], F32, kind="ExternalInput").ap()
    wvt_d = nc.dram_tensor("wvt", [P, NPAIR * D], F32, kind="ExternalInput").ap()
    bv_d = nc.dram_tensor("bv", [P, NPAIR], F32, kind="ExternalInput").ap()
    ones_d = nc.dram_tensor("ones", [1, P], F32, kind="ExternalInput").ap()
    out_d = nc.dram_tensor("out", [NSEQ, D], F32, kind="ExternalOutput").ap()

    with tile.TileContext(nc) as tc:
        with (
            tc.tile_pool(name="const", bufs=1) as cpool,
            tc.tile_pool(name="xt", bufs=xt_bufs) as xtpool,
            tc.tile_pool(name="qk", bufs=2) as qkpool,
            tc.tile_pool(name="wv", bufs=1) as wvpool,
            tc.tile_pool(name="osb", bufs=2) as opool,
            tc.tile_pool(name="sm", bufs=1) as smpool,
            tc.tile_pool(name="psmm", bufs=2, space="PSUM") as ps_mm_pool,
            tc.tile_pool(name="psbig", bufs=1, space="PSUM") as ps_big_pool,
        ):
            # ---- constants ----
            WQK = 2 * D
            w_sb = cpool.tile([P, KT * WQK], F32R, tag="w")
            w_v = w_sb[:].rearrange("p (t n) -> p t n", t=KT)
            w_dv = w_d.rearrange("(t p) n -> p t n", p=P)
            # chunk-0 xT first so QK(m0) can start as soon as W[0] lands
            xT0 = xtpool.tile([P, KT * CHUNK], F32R, tag="xT")
            nc.gpsimd.dma_start(
                xT0[:].rearrange("p (t r) -> p t r", t=KT),
                xt_d.rearrange("(t p) r -> p t r", p=P)[:, :, :CHUNK],
            )
            for t in range(KT):  # per k-tile so chunk-0 QK pipelines
                nc.gpsimd.dma_start(
                    w_v[:, t : t + 1, :], w_dv[:, t : t + 1, : 2 * D]
                )
            bqk = cpool.tile([P, 2 * D], F32, tag="bqk")
            nc.sync.dma_start(bqk[:], bqk_d)
            # WvT packed [128 = v-channel within pair, pair-major x D-rows]
            # (loaded late - only needed after softmax)
            wvt = cpool.tile([P, NPAIR * D], F32R, tag="wvt")
            bv = cpool.tile([P, NPAIR], F32R, tag="bv")
            ones = cpool.tile([1, P], F32R, tag="ones")
            nc.gpsimd.dma_start(ones[:], ones_d)
            bd = cpool.tile([P, NPAIR * P], F32R, tag="bd")
            # Wv' = Wv @ attn (natural layout, D-rows on k-tiles) + bv' row
            wvp = wvpool.tile([P, KT * D], F32R, tag="wvp")
            bvp = wvpool.tile([1, D], F32R, tag="bvp")

            # attention accumulator in SBUF (pair p at cols [128p..])
            attn_acc = cpool.tile([P, NPAIR * P], F32, tag="attn_acc")

            # ================= Pass A: QK projection + attn + V spill
            for ch in range(NCHUNK):
                if ch == 0:
                    xT_sb = xT0
                else:
                    xT_sb = xtpool.tile([P, KT * CHUNK], F32R, tag="xT")
                    nc.gpsimd.dma_start(
                        xT_sb[:].rearrange("p (t r) -> p t r", t=KT),
                        xt_d.rearrange("(t p) r -> p t r", p=P)[
                            :, :, ch * CHUNK : (ch + 1) * CHUNK
                        ],
                    )
                if ch == 1:
                    nc.gpsimd.dma_start(wvt[:], wvt_d)
                    nc.gpsimd.dma_start(bv[:], bv_d)
                for mi in range(MPC):
                    m = ch * MPC + mi
                    qk_sb = qkpool.tile([P, 2 * D], F32R, tag="qk")
                    for nh in range(2):  # halves of the 2048 qk cols
                        ps_qk = ps_mm_pool.tile([P, D], F32, tag="mm")
                        for k in range(KT):
                            for n2 in range(2):  # same stationary xT[k]
                                n = nh * 2 + n2
                                nc.tensor.matmul(
                                    ps_qk[:, bass.ts(n2, 512)],
                                    xT_sb[:, CHUNK * k + mi * P : CHUNK * k + (mi + 1) * P],
                                    w_sb[:, WQK * k + 512 * n : WQK * k + 512 * (n + 1)],
                                    start=(k == 0),
                                    stop=(k == KT - 1),
                                )
                        nc.vector.tensor_add(
                            qk_sb[:, bass.ts(nh, D)],
                            ps_qk[:],
                            bqk[:, bass.ts(nh, D)],
                        )
                    # attention partials per head pair; rhs spans TWO
                    # pairs' k-cols (N=256) so the f32r matmul runs at
                    # 1 cyc/row instead of the <256-wide 4x penalty; the
                    # junk half is dropped at extraction
                    ps_attn = ps_big_pool.tile([P, 2 * NPAIR * P], F32, tag="big")
                    for p in range(NPAIR):
                        j = p // 2
                        nc.tensor.matmul(
                            ps_attn[:, 256 * p : 256 * (p + 1)],
                            qk_sb[:, bass.ts(p, P)],
                            qk_sb[:, D + 256 * j : D + 256 * (j + 1)],
                            start=True,
                            stop=True,
                        )
                    # even pairs: useful at ps[512j + 0] -> acc[256j + 0]
                    # odd pairs:  useful at ps[512j + 384] -> acc[256j + 128]
                    ps_j = ps_attn[:].rearrange("q (j t) -> q j t", j=4)
                    ac_j = attn_acc[:].rearrange("q (j t) -> q j t", j=4)
                    for par in range(2):
                        src = ps_j[:, :, 384 * par : 384 * par + P]
                        dst = ac_j[:, :, P * par : P * (par + 1)]
                        if m == 0:
                            nc.vector.tensor_copy(dst, src)
                        else:
                            nc.vector.tensor_add(dst, dst, src)
            # ================= Softmax (batched over pairs, diag blocks)
            # upper half: partitions 0:64 use cols [128p, 128p+64)
            # lower half: partitions 64:128 use cols [128p+64, 128p+128)
            negmax = smpool.tile([P, NPAIR], F32, tag="negmax")
            shifted = smpool.tile([P, NPAIR * DH], F32, tag="shifted")
            expv = shifted
            rsum = smpool.tile([P, NPAIR], F32, tag="rsum")
            rinv = smpool.tile([P, NPAIR], F32, tag="rinv")
            nc.vector.tensor_scalar_mul(bd[:], attn_acc[:], 0.0)
            for half in range(2):
                pr = slice(half * DH, (half + 1) * DH)
                att_v = attn_acc[pr].rearrange("q (p e) -> q p e", p=NPAIR)[
                    :, :, half * DH : (half + 1) * DH
                ]
                nc.vector.reduce_max(
                    negmax[pr, :], att_v, axis=mybir.AxisListType.X, negate=True
                )
                sh_v = shifted[pr].rearrange("q (p e) -> q p e", p=NPAIR)
                nm_b = negmax[pr, :].broadcast_to([DH, NPAIR, DH])
                nc.vector.tensor_add(sh_v, att_v, nm_b)
                nc.scalar.activation(
                    expv[pr, :], shifted[pr, :],
                    mybir.ActivationFunctionType.Exp,
                    scale=0.125,
                )
                ex_v = expv[pr].rearrange("q (p e) -> q p e", p=NPAIR)
                nc.vector.reduce_sum(
                    rsum[pr, :], ex_v, axis=mybir.AxisListType.X
                )
                nc.vector.reciprocal(rinv[pr, :], rsum[pr, :])
                bd_v = bd[pr].rearrange("q (p e) -> q p e", p=NPAIR)[
                    :, :, half * DH : (half + 1) * DH
                ]
                ri_b = rinv[pr, :].broadcast_to([DH, NPAIR, DH])
                nc.vector.tensor_mul(bd_v, ex_v, ri_b)

            # ---- Wv' = Wv @ attn via wide matmuls (junk halves dropped)
            # Wv'[Drow, e-col]; stored per D-k-tile at wvp[:, 1024*t ..]
            for t in range(KT):
                ps_wv = ps_big_pool.tile([P, 2 * NPAIR * P], F32, tag="big")
                for p in range(NPAIR):
                    j = p // 2
                    nc.tensor.matmul(
                        ps_wv[:, 256 * p : 256 * (p + 1)],
                        wvt[:, D * p + t * P : D * p + (t + 1) * P],
                        bd[:, 256 * j : 256 * (j + 1)],
                        start=True,
                        stop=True,
                    )
                ps_j = ps_wv[:].rearrange("q (j u) -> q j u", j=4)
                wv_j = wvp[:, bass.ts(t, D)].rearrange("q (j u) -> q j u", j=4)
                for par in range(2):
                    nc.vector.tensor_copy(
                        wv_j[:, :, P * par : P * (par + 1)],
                        ps_j[:, :, 384 * par : 384 * par + P],
                    )
            # bv' = bv @ attn: per pair K=128 matmul with M=1
            ps_bv = ps_mm_pool.tile([1, D], F32, tag="mm")
            for p in range(NPAIR):
                j = p // 2
                nc.tensor.matmul(
                    ps_bv[:, bass.ts(p, P)],
                    bv[:, p : p + 1],
                    bd[:, 256 * j + 128 * (p % 2) : 256 * j + 128 * (p % 2) + P],
                    start=True,
                    stop=True,
                )
            nc.vector.tensor_copy(bvp[:], ps_bv[:])

            # ================= Pass B: out = x @ Wv' + bv' (third
            # projection; re-streams xT, PE-dense, no spill)
            for ch in range(NCHUNK):
                xT_sb = xtpool.tile([P, KT * CHUNK], F32R, tag="xT")
                nc.gpsimd.dma_start(
                    xT_sb[:].rearrange("p (t r) -> p t r", t=KT),
                    xt_d.rearrange("(t p) r -> p t r", p=P)[
                        :, :, ch * CHUNK : (ch + 1) * CHUNK
                    ],
                )
                for mi in range(MPC):
                    m = ch * MPC + mi
                    ps_o = ps_mm_pool.tile([P, D], F32, tag="mm")
                    for k in range(KT):
                        for n2 in range(2):
                            nc.tensor.matmul(
                                ps_o[:, bass.ts(n2, 512)],
                                xT_sb[:, CHUNK * k + mi * P : CHUNK * k + (mi + 1) * P],
                                wvp[:, D * k + 512 * n2 : D * k + 512 * (n2 + 1)],
                                start=(k == 0),
                                stop=False,
                            )
                    for n2 in range(2):  # rank-1 bias fold, closes group
                        nc.tensor.matmul(
                            ps_o[:, bass.ts(n2, 512)],
                            ones[:],
                            bvp[:, bass.ts(n2, 512)],
                            start=False,
                            stop=True,
                        )
                    out_sb = opool.tile([P, D], F32, tag="osb")
                    nc.vector.tensor_copy(out_sb[:], ps_o[:])
                    nc.scalar.dma_start(
                        out_d[m * P : (m + 1) * P, :], out_sb[:]
                    )

    nc.compile()
    return nc


def kernel(x, W_qkv, b_qkv):
    global _LAST_RESULTS
    x = np.ascontiguousarray(x, dtype=np.float32)
    W_qkv = np.ascontiguousarray(W_qkv, dtype=np.float32)
    b_qkv = np.ascontiguousarray(b_qkv, dtype=np.float32)

    if "nc" not in _CACHE:
        _CACHE["nc"] = _build()
    nc = _CACHE["nc"]

    bqk = np.broadcast_to(b_qkv[: 2 * D][None, :], (P, 2 * D)).copy()
    # WvT packed: [128 = v-channel within pair, pair-major x D-rows]
    wvt = np.ascontiguousarray(
        W_qkv[:, 2 * D :].T.reshape(NPAIR, P, D).transpose(1, 0, 2)
        .reshape(P, NPAIR * D)
    )
    bv = np.ascontiguousarray(b_qkv[2 * D :].reshape(NPAIR, P).T)
    ones = np.ones((1, P), np.float32)

    in_maps = [
        {
            "xt": np.ascontiguousarray(x[c].T),
            "w": W_qkv,
            "bqk": bqk,
            "wvt": wvt,
            "bv": bv,
            "ones": ones,
        }
        for c in range(B)
    ]
    res = bass_utils.run_bass_kernel_spmd(
        nc, in_maps, core_ids=list(range(B))
    )
    _LAST_RESULTS = res
    return np.stack([r["out"] for r in res.results], axis=0)


# revision 33
# speedup vs baseline: 1.0074x; 1.0074x over previous
"""BNT Channel Attention kernel for 8x TRN2 NeuronCores.

Reference computation (per batch b of 8, one batch per core):
    qkv = x @ W_qkv + b_qkv            # [4096, 3072]
    q, k, v = split(qkv)               # each [4096, 1024], 16 heads x 64
    attn_h = softmax((q_h^T @ k_h) / 8, axis=-1)   # [64, 64] per head
    out_h  = v_h @ attn_h              # [4096, 64]
    out    = concat_h(out_h)           # [4096, 1024]

Strategy:
- Data parallel over batch: core c handles batch c (no collectives).
- All matmuls in float32r (fp32 storage, 11-bit-mantissa PE mode, 1 cyc/row
  vs 4 for fp32). End-to-end absmax-rel error vs the f32 reference ~1.2e-3.
- The projection contracts over D, so x must be D-on-partitions: x is
  transposed on the HOST (numpy) and fed as xt [1024, 4096]; on-chip it is
  streamed per 512-row chunk with a single cast-DMA (f32 -> f32r).
- Pass A per chunk: QK projection (N=512 matmuls, PSUM k-accumulation),
  per-head-pair attention partials (closed per row-tile, accumulated in
  SBUF by DVE - long-lived PSUM accumulation is unsafe under the Tile
  scheduler). Attention matmuls are N=256 wide (two pairs of k-cols,
  junk half dropped) to dodge the f32r <256-wide 4x weight-load penalty.
- Softmax: batched over all 8 head pairs, writes a block-diagonal
  [128, 8*128] f32r tile bd (pair p diag blocks at cols 128p..).
- V path by associativity: out = x @ (Wv @ attn) + (bv @ attn).
  Wv' = Wv @ attn is tiny (host-pretransposed WvT input); pass B is then
  a plain second projection out = x @ Wv' re-streaming xT (PE-dense, no
  vT spill), with the bias folded in as rank-1 ones x bv' matmuls.
"""

import numpy as np

import concourse.bacc as bacc
import concourse.bass as bass
import concourse.mybir as mybir
import concourse.tile as tile
from concourse import bass_utils

B = 8
NSEQ = 4096
D = 1024
ND3 = 3 * D
H = 16
DH = 64
NPAIR = 8          # head pairs (2 heads = 128 channels per pair)
P = 128            # partitions / tile edge
KT = D // P        # 8 k-tiles over the contraction dim
MT = NSEQ // P     # 32 row-tiles
CHUNK = 512        # rows per chunk
NCHUNK = NSEQ // CHUNK
MPC = CHUNK // P   # 4 row-tiles per chunk

F32 = mybir.dt.float32
F32R = mybir.dt.float32r

_CACHE = {}
_LAST_RESULTS = None


def _build(xt_bufs=2):
    nc = bacc.Bacc(
        "TRN2", target_bir_lowering=False, debug=False, num_devices=B
    )
    xt_d = nc.dram_tensor("xt", [D, NSEQ], F32, kind="ExternalInput").ap()
    w_d = nc.dram_tensor("w", [D, ND3], F32, kind="ExternalInput").ap()
    bqk_d = nc.dram_tensor("bqk", [P, 2 * D], F32, kind="ExternalInput").ap()
    wvt_d = nc.dram_tensor("wvt", [P, NPAIR * D], F32, kind="ExternalInput").ap()
    bv_d = nc.dram_tensor("bv", [P, NPAIR], F32, kind="ExternalInput").ap()
    ones_d = nc.dram_tensor("ones", [1, P], F32, kind="ExternalInput").ap()
    out_d = nc.dram_tensor("out", [NSEQ, D], F32, kind="ExternalOutput").ap()

    with tile.TileContext(nc) as tc:
        with (
            tc.tile_pool(name="const", bufs=1) as cpool,
            tc.tile_pool(name="xt", bufs=xt_bufs) as xtpool,
            tc.tile_pool(name="qk", bufs=2) as qkpool,
            tc.tile_pool(name="wv", bufs=1) as wvpool,
            tc.tile_pool(name="osb", bufs=2) as opool,
            tc.tile_pool(name="sm", bufs=1) as smpool,
            tc.tile_pool(name="psmm", bufs=2, space="PSUM") as ps_mm_pool,
            tc.tile_pool(name="psbig", bufs=1, space="PSUM") as ps_big_pool,
        ):
            # ---- constants ----
            WQK = 2 * D
            w_sb = cpool.tile([P, KT * WQK], F32R, tag="w")
            w_v = w_sb[:].rearrange("p (t n) -> p t n", t=KT)
            w_dv = w_d.rearrange("(t p) n -> p t n", p=P)
            # chunk-0 xT first so QK(m0) can start as soon as W[0] lands
            xT0 = xtpool.tile([P, KT * CHUNK], F32R, tag="xT")
            nc.gpsimd.dma_start(
                xT0[:].rearrange("p (t r) -> p t r", t=KT),
                xt_d.rearrange("(t p) r -> p t r", p=P)[:, :, :CHUNK],
            )
            for t in range(KT):  # per k-tile so chunk-0 QK pipelines
                nc.gpsimd.dma_start(
                    w_v[:, t : t + 1, :], w_dv[:, t : t + 1, : 2 * D]
                )
            bqk = cpool.tile([P, 2 * D], F32, tag="bqk")
            nc.sync.dma_start(bqk[:], bqk_d)
            # WvT packed [128 = v-channel within pair, pair-major x D-rows]
            # (loaded late - only needed after softmax)
            wvt = cpool.tile([P, NPAIR * D], F32R, tag="wvt")
            bv = cpool.tile([P, NPAIR], F32R, tag="bv")
            ones = cpool.tile([1, P], F32R, tag="ones")
            nc.gpsimd.dma_start(ones[:], ones_d)
            bd = cpool.tile([P, NPAIR * P], F32R, tag="bd")
            # Wv' = Wv @ attn (natural layout, D-rows on k-tiles) + bv' row
            wvp = wvpool.tile([P, KT * D], F32R, tag="wvp")
            bvp = wvpool.tile([1, D], F32R, tag="bvp")

            # attention accumulator in SBUF (pair p at cols [128p..])
            attn_acc = cpool.tile([P, NPAIR * P], F32, tag="attn_acc")

            # ================= Pass A: QK projection + attn + V spill
            for ch in range(NCHUNK):
                if ch == 0:
                    xT_sb = xT0
                else:
                    xT_sb = xtpool.tile([P, KT * CHUNK], F32R, tag="xT")
                    nc.gpsimd.dma_start(
                        xT_sb[:].rearrange("p (t r) -> p t r", t=KT),
                        xt_d.rearrange("(t p) r -> p t r", p=P)[
                            :, :, ch * CHUNK : (ch + 1) * CHUNK
                        ],
                    )
                if ch == 1:
                    nc.gpsimd.dma_start(wvt[:], wvt_d)
                    nc.gpsimd.dma_start(bv[:], bv_d)
                for mi in range(MPC):
                    m = ch * MPC + mi
                    qk_sb = qkpool.tile([P, 2 * D], F32R, tag="qk")
                    for nh in range(2):  # halves of the 2048 qk cols
                        ps_qk = ps_mm_pool.tile([P, D], F32, tag="mm")
                        for k in range(KT):
                            for n2 in range(2):  # same stationary xT[k]
                                n = nh * 2 + n2
                                nc.tensor.matmul(
                                    ps_qk[:, bass.ts(n2, 512)],
                                    xT_sb[:, CHUNK * k + mi * P : CHUNK * k + (mi + 1) * P],
                                    w_sb[:, WQK * k + 512 * n : WQK * k + 512 * (n + 1)],
                                    start=(k == 0),
                                    stop=(k == KT - 1),
                                )
                        nc.vector.tensor_add(
                            qk_sb[:, bass.ts(nh, D)],
                            ps_qk[:],
                            bqk[:, bass.ts(nh, D)],
                        )
                    # attention partials per head pair; rhs spans TWO
                    # pairs' k-cols (N=256) so the f32r matmul runs at
                    # 1 cyc/row instead of the <256-wide 4x penalty; the
                    # junk half is dropped at extraction
                    ps_attn = ps_big_pool.tile([P, 2 * NPAIR * P], F32, tag="big")
                    for p in range(NPAIR):
                        j = p // 2
                        nc.tensor.matmul(
                            ps_attn[:, 256 * p : 256 * (p + 1)],
                            qk_sb[:, bass.ts(p, P)],
                            qk_sb[:, D + 256 * j : D + 256 * (j + 1)],
                            start=True,
                            stop=True,
                        )
                    # even pairs: useful at ps[512j + 0] -> acc[256j + 0]
                    # odd pairs:  useful at ps[512j + 384] -> acc[256j + 128]
                    ps_j = ps_attn[:].rearrange("q (j t) -> q j t", j=4)
                    ac_j = attn_acc[:].rearrange("q (j t) -> q j t", j=4)
                    for par in range(2):
                        src = ps_j[:, :, 384 * par : 384 * par + P]
                        dst = ac_j[:, :, P * par : P * (par + 1)]
                        if m == 0:
                            nc.vector.tensor_copy(dst, src)
                        else:
                            nc.vector.tensor_add(dst, dst, src)
            # ================= Softmax (batched over pairs, diag blocks)
            # upper half: partitions 0:64 use cols [128p, 128p+64)
            # lower half: partitions 64:128 use cols [128p+64, 128p+128)
            negmax = smpool.tile([P, NPAIR], F32, tag="negmax")
            shifted = smpool.tile([P, NPAIR * DH], F32, tag="shifted")
            expv = shifted
            rsum = smpool.tile([P, NPAIR], F32, tag="rsum")
            rinv = smpool.tile([P, NPAIR], F32, tag="rinv")
            nc.vector.tensor_scalar_mul(bd[:], attn_acc[:], 0.0)
            for half in range(2):
                pr = slice(half * DH, (half + 1) * DH)
                att_v = attn_acc[pr].rearrange("q (p e) -> q p e", p=NPAIR)[
                    :, :, half * DH : (half + 1) * DH
                ]
                nc.vector.reduce_max(
                    negmax[pr, :], att_v, axis=mybir.AxisListType.X, negate=True
                )
                sh_v = shifted[pr].rearrange("q (p e) -> q p e", p=NPAIR)
                nm_b = negmax[pr, :].broadcast_to([DH, NPAIR, DH])
                nc.vector.tensor_add(sh_v, att_v, nm_b)
                nc.scalar.activation(
                    expv[pr, :], shifted[pr, :],
                    mybir.ActivationFunctionType.Exp,
                    scale=0.125,
                )
                ex_v = expv[pr].rearrange("q (p e) -> q p e", p=NPAIR)
                nc.vector.reduce_sum(
                    rsum[pr, :], ex_v, axis=mybir.AxisListType.X
                )
                nc.vector.reciprocal(rinv[pr, :], rsum[pr, :])
                bd_v = bd[pr].rearrange("q (p e) -> q p e", p=NPAIR)[
                    :, :, half * DH : (half + 1) * DH
                ]
                ri_b = rinv[pr, :].broadcast_to([DH, NPAIR, DH])
                nc.vector.tensor_mul(bd_v, ex_v, ri_b)

            # ---- Wv' = Wv @ attn via wide matmuls (junk halves dropped)
            # Wv'[Drow, e-col]; stored per D-k-tile at wvp[:, 1024*t ..]
            for t in range(KT):
                ps_wv = ps_big_pool.tile([P, 2 * NPAIR * P], F32, tag="big")
                for p in range(NPAIR):
                    j = p // 2
                    nc.tensor.matmul(
                        ps_wv[:, 256 * p : 256 * (p + 1)],
                        wvt[:, D * p + t * P : D * p + (t + 1) * P],
                        bd[:, 256 * j : 256 * (j + 1)],
                        start=True,
                        stop=True,
                    )
                ps_j = ps_wv[:].rearrange("q (j u) -> q j u", j=4)
                wv_j = wvp[:, bass.ts(t, D)].rearrange("q (j u) -> q j u", j=4)
                for par in range(2):
                    nc.vector.tensor_copy(
                        wv_j[:, :, P * par : P * (par + 1)],
                        ps_j[:, :, 384 * par : 384 * par + P],
                    )
            # bv' = bv @ attn: per pair K=128 matmul with M=1
            ps_bv = ps_mm_pool.tile([1, D], F32, tag="mm")
            for p in range(NPAIR):
                j = p // 2
                nc.tensor.matmul(
                    ps_bv[:, bass.ts(p, P)],
                    bv[:, p : p + 1],
                    bd[:, 256 * j + 128 * (p % 2) : 256 * j + 128 * (p % 2) + P],
                    start=True,
                    stop=True,
                )
            nc.vector.tensor_copy(bvp[:], ps_bv[:])
            # replicate bv' across partitions once into attn_acc's storage
            # (f32, dead after softmax, never matmul-consumed); pass-B out
            # copies become adds, dropping 64 rank-1 matmuls from PE
            ps_br = ps_mm_pool.tile([P, D], F32, tag="mm")
            for n2 in range(2):
                nc.tensor.matmul(
                    ps_br[:, bass.ts(n2, 512)],
                    ones[:],
                    bvp[:, bass.ts(n2, 512)],
                    start=True,
                    stop=True,
                )
            nc.vector.tensor_copy(attn_acc[:], ps_br[:])

            # ================= Pass B: out = x @ Wv' + bv' (third
            # projection; re-streams xT, PE-dense, no spill)
            for ch in range(NCHUNK):
                xT_sb = xtpool.tile([P, KT * CHUNK], F32R, tag="xT")
                nc.gpsimd.dma_start(
                    xT_sb[:].rearrange("p (t r) -> p t r", t=KT),
                    xt_d.rearrange("(t p) r -> p t r", p=P)[
                        :, :, ch * CHUNK : (ch + 1) * CHUNK
                    ],
                )
                for mi in range(MPC):
                    m = ch * MPC + mi
                    ps_o = ps_mm_pool.tile([P, D], F32, tag="mm")
                    for k in range(KT):
                        for n2 in range(2):
                            nc.tensor.matmul(
                                ps_o[:, bass.ts(n2, 512)],
                                xT_sb[:, CHUNK * k + mi * P : CHUNK * k + (mi + 1) * P],
                                wvp[:, D * k + 512 * n2 : D * k + 512 * (n2 + 1)],
                                start=(k == 0),
                                stop=(k == KT - 1),
                            )
                    out_sb = opool.tile([P, D], F32, tag="osb")
                    nc.vector.tensor_add(out_sb[:], ps_o[:], attn_acc[:])
                    nc.scalar.dma_start(
                        out_d[m * P : (m + 1) * P, :], out_sb[:]
                    )

    nc.compile()
    return nc


def kernel(x, W_qkv, b_qkv):
    global _LAST_RESULTS
    x = np.ascontiguousarray(x, dtype=np.float32)
    W_qkv = np.ascontiguousarray(W_qkv, dtype=np.float32)
    b_qkv = np.ascontiguousarray(b_qkv, dtype=np.float32)

    if "nc" not in _CACHE:
        _CACHE["nc"] = _build()
    nc = _CACHE["nc"]

    bqk = np.broadcast_to(b_qkv[: 2 * D][None, :], (P, 2 * D)).copy()
    # WvT packed: [128 = v-channel within pair, pair-major x D-rows]
    wvt = np.ascontiguousarray(
        W_qkv[:, 2 * D :].T.reshape(NPAIR, P, D).transpose(1, 0, 2)
        .reshape(P, NPAIR * D)
    )
    bv = np.ascontiguousarray(b_qkv[2 * D :].reshape(NPAIR, P).T)
    ones = np.ones((1, P), np.float32)

    in_maps = [
        {
            "xt": np.ascontiguousarray(x[c].T),
            "w": W_qkv,
            "bqk": bqk,
            "wvt": wvt,
            "bv": bv,
            "ones": ones,
        }
        for c in range(B)
    ]
    res = bass_utils.run_bass_kernel_spmd(
        nc, in_maps, core_ids=list(range(B))
    )
    _LAST_RESULTS = res
    return np.stack([r["out"] for r in res.results], axis=0)
